# revision 3
# baseline (speedup 1.0000x reference)
"""Trainium2 Bass kernel for nn_CNN2DImplemented_51994874085714.

conv2d: x (16, 64, 112, 112) f32 * weight (64, 3, 3, 128) -> (16, 128, 112, 112),
3x3, pad=1, stride=1 (weight layout (C_in, kh, kw, C_out), no bias).

Sharding: data-parallel over batch - 2 images per NeuronCore on 8 cores,
weight replicated; each core computes its shard independently (no
collectives) and the host concatenates the per-core outputs.

Per-core kernel (implicit GEMM, fp16 in / fp16 out, converted on host):
the PE runs in 64x128 row-tiled mode - the 128x128 array splits into two
independent 64-row tiles (T0 = SBUF partitions 0:64, T8 = 64:128). Each
half holds one 28-output-row strip of a different image (C=64 channels in
the partition dim); the 9 filter taps are 9 accumulating K=64 matmuls per
4-row output group (N<=448), running concurrently on both halves into
separate PSUM banks, which keeps all 128 PE rows busy with no wasted
zero-padded weight halves and no shifted x copies.

Zero padding is never materialized: x strips are stored unpadded (one
contiguous DMA descriptor per partition) and boundary taps shrink the
matmul's output column/row range - PSUM's per-element has_written bit makes
the first writer of each element overwrite, so partial-range accumulation
is exact.

Engine split: x loads on the sync HWDGE ring, weight + output stores on the
scalar HWDGE ring; VectorE evacuates T0's PSUM banks, ScalarE T8's. Dummy
matmuls on constant SBUF data warm the PE clock (HAM throttle) during the
initial x DMA wait.
"""

from contextlib import ExitStack

import numpy as np

N_CORES = 8
B, C, H, W, O = 16, 64, 112, 112, 128
B_LOC = B // N_CORES
S = 28  # output rows per strip

_cache = {}


def _build_nc():
    import concourse.mybir as mybir
    import concourse.tile as tile
    from concourse import bacc

    F16 = mybir.dt.float16
    F32 = mybir.dt.float32

    NS = H // S          # strips per image
    R = S + 2            # x rows per strip incl. halo (rows 0/R-1 unused at edges)

    nc = bacc.Bacc("TRN2", target_bir_lowering=False, debug=False,
                   num_devices=1)
    x_d = nc.declare_dram_parameter("x", [B_LOC, C, H, W], F16, isOutput=False)
    w_d = nc.declare_dram_parameter("weight", [C, 3, 3, O], F16, isOutput=False)
    o_d = nc.declare_dram_parameter("out", [B_LOC, O, H, W], F16, isOutput=True)

    with tile.TileContext(nc) as tc, ExitStack() as ctx:
        wpool = ctx.enter_context(tc.tile_pool(name="weights", bufs=1))
        xpool = ctx.enter_context(tc.tile_pool(name="xstrips", bufs=3))
        spool = ctx.enter_context(tc.tile_pool(name="staging", bufs=4))
        ppool = ctx.enter_context(tc.tile_pool(name="psum", bufs=4, space="PSUM"))

        # PE warm-up during the initial DMA wait: two interleaved chains of
        # dummy matmuls (one per 64-row tile) on constant SBUF data (non-zero,
        # so the activity monitor sees real switching). Their PSUM tiles come
        # from the same pool tags as the real groups, so the banks are
        # recycled once the dummies are long done.
        dummy = wpool.tile([128, 512], F16, tag="dummy")
        nc.vector.memset(dummy[:, :], 1.0)
        dps0 = ppool.tile([O, 4, W], F32, tag="ps0")
        dps1 = ppool.tile([O, 4, W], F32, tag="ps1")
        for _ in range(14):
            nc.tensor.matmul(dps0[:, :, :], dummy[0:64, 0:O],
                             dummy[0:64, 0:4 * W], start=True, stop=True)
            nc.tensor.matmul(dps1[:, :, :], dummy[64:128, 0:O],
                             dummy[64:128, 0:4 * W], start=True, stop=True)

        # All 9 filter taps in one [128, 9, O] tile, one DMA per half on the
        # scalar HWDGE ring (x loads use the sync ring concurrently).
        wt = wpool.tile([128, 9, O], F16, tag="wt")
        wflat = w_d[:, :, :, :]
        nc.scalar.dma_start(wt[0:64, :, :], wflat)
        nc.scalar.dma_start(wt[64:128, :, :], wflat)

        taps = [(dh, dw) for dh in range(3) for dw in range(3)]

        def load_pair(s, split=False):
            h0 = s * S
            xb = xpool.tile([128, R, W], F16, tag="xs")
            r_lo = 1 if s == 0 else 0
            r_hi = R - 1 if s == NS - 1 else R
            if split:
                # land the first groups' rows early so compute starts sooner
                for a, b in ((r_lo, 8), (8, 20), (20, r_hi)):
                    nc.sync.dma_start(
                        xb[:, a:b, :],
                        x_d[:, :, h0 + a - 1:h0 + b - 1, :],
                    )
            else:
                nc.sync.dma_start(
                    xb[:, r_lo:r_hi, :],
                    x_d[:, :, h0 + r_lo - 1:h0 + r_hi - 1, :],
                )
            return xb

        def compute_pair(s, xb):
            h0 = s * S
            stg0 = spool.tile([O, S, W], F16, tag="stg0")
            stg1 = spool.tile([O, S, W], F16, tag="stg1")
            for j in range(S // 4):
                l0 = 4 * j
                ps0 = ppool.tile([O, 4, W], F32, tag="ps0")
                ps1 = ppool.tile([O, 4, W], F32, tag="ps1")
                for t, (dh, dw) in enumerate(taps):
                    st = t == 0
                    sp = t == 8
                    # output cols valid for this dw (x col = out col + dw - 1)
                    oc0, oc1 = (1, W) if dw == 0 else (0, W - 1) if dw == 2 else (0, W)
                    xc0 = oc0 + dw - 1
                    # output rows valid for this dh (tile row = out row + dh)
                    i0 = 1 if (s == 0 and l0 + dh < 1) else 0
                    i1 = min(4, (R - 1) - l0 - dh) if s == NS - 1 else 4
                    nc.tensor.matmul(
                        ps0[:, i0:i1, oc0:oc1], wt[0:64, t, :],
                        xb[0:64, l0 + dh + i0:l0 + dh + i1, xc0:xc0 + oc1 - oc0],
                        start=st, stop=sp,
                    )
                    nc.tensor.matmul(
                        ps1[:, i0:i1, oc0:oc1], wt[64:128, t, :],
                        xb[64:128, l0 + dh + i0:l0 + dh + i1, xc0:xc0 + oc1 - oc0],
                        start=st, stop=sp,
                    )
                nc.vector.tensor_copy(stg0[:, l0:l0 + 4, :], ps0[:, :, :])
                nc.scalar.copy(stg1[:, l0:l0 + 4, :], ps1[:, :, :])
                # stream out finished rows while the rest computes
                if j == 2:
                    nc.sync.dma_start(o_d[0, :, h0:h0 + 12, :],
                                      stg0[:, 0:12, :])
                    nc.scalar.dma_start(o_d[1, :, h0:h0 + 12, :],
                                        stg1[:, 0:12, :])
                elif j == 5:
                    nc.sync.dma_start(o_d[0, :, h0 + 12:h0 + 24, :],
                                      stg0[:, 12:24, :])
                    nc.scalar.dma_start(o_d[1, :, h0 + 12:h0 + 24, :],
                                        stg1[:, 12:24, :])
            nc.sync.dma_start(o_d[0, :, h0 + 24:h0 + S, :], stg0[:, 24:S, :])
            nc.scalar.dma_start(o_d[1, :, h0 + 24:h0 + S, :], stg1[:, 24:S, :])

        cur = load_pair(0, split=True)
        for s in range(NS):
            nxt = load_pair(s + 1) if s + 1 < NS else None
            compute_pair(s, cur)
            cur = nxt

    nc.compile()
    return nc


def kernel(x: np.ndarray, weight: np.ndarray) -> np.ndarray:
    from concourse.bass_utils import run_bass_kernel_spmd

    if "nc" not in _cache:
        _cache["nc"] = _build_nc()
    nc = _cache["nc"]

    x16 = np.ascontiguousarray(np.asarray(x).astype(np.float16))
    w16 = np.ascontiguousarray(np.asarray(weight).astype(np.float16))

    in_maps = [
        {"x": x16[i * B_LOC:(i + 1) * B_LOC], "weight": w16}
        for i in range(N_CORES)
    ]
    res = run_bass_kernel_spmd(nc, in_maps, list(range(N_CORES)))
    return np.concatenate(
        [np.asarray(res.results[i]["out"], dtype=np.float32)
         for i in range(N_CORES)],
        axis=0,
    )


# revision 4
# speedup vs baseline: 1.0174x; 1.0174x over previous
"""Trainium2 Bass kernel for nn_CNN2DImplemented_51994874085714.

conv2d: x (16, 64, 112, 112) f32 * weight (64, 3, 3, 128) -> (16, 128, 112, 112),
3x3, pad=1, stride=1 (weight layout (C_in, kh, kw, C_out), no bias).

Sharding: data-parallel over batch - 2 images per NeuronCore on 8 cores,
weight replicated; each core computes its shard independently (no
collectives) and the host concatenates the per-core outputs.

Per-core kernel (implicit GEMM, fp16 in / fp16 out, converted on host):
the PE runs in 64x128 row-tiled mode - the 128x128 array splits into two
independent 64-row tiles (T0 = SBUF partitions 0:64, T8 = 64:128). Each
half holds one 28-output-row strip of a different image (C=64 channels in
the partition dim); the 9 filter taps are 9 accumulating K=64 matmuls per
4-row output group (N<=448), running concurrently on both halves into
separate PSUM banks, which keeps all 128 PE rows busy with no wasted
zero-padded weight halves and no shifted x copies.

Zero padding is never materialized: x strips are stored unpadded (one
contiguous DMA descriptor per partition) and boundary taps shrink the
matmul's output column/row range - PSUM's per-element has_written bit makes
the first writer of each element overwrite, so partial-range accumulation
is exact.

Engine split: x loads on the sync HWDGE ring, weight + output stores on the
scalar HWDGE ring; VectorE evacuates T0's PSUM banks, ScalarE T8's. Dummy
matmuls on constant SBUF data warm the PE clock (HAM throttle) during the
initial x DMA wait.
"""

from contextlib import ExitStack

import numpy as np

N_CORES = 8
B, C, H, W, O = 16, 64, 112, 112, 128
B_LOC = B // N_CORES
S = 28  # output rows per strip

_cache = {}


def _build_nc():
    import concourse.mybir as mybir
    import concourse.tile as tile
    from concourse import bacc

    F16 = mybir.dt.float16
    F32 = mybir.dt.float32

    NS = H // S          # strips per image
    R = S + 2            # x rows per strip incl. halo (rows 0/R-1 unused at edges)

    nc = bacc.Bacc("TRN2", target_bir_lowering=False, debug=False,
                   num_devices=1)
    x_d = nc.declare_dram_parameter("x", [B_LOC, C, H, W], F16, isOutput=False)
    w_d = nc.declare_dram_parameter("weight", [C, 3, 3, O], F16, isOutput=False)
    o_d = nc.declare_dram_parameter("out", [B_LOC, O, H, W], F16, isOutput=True)

    with tile.TileContext(nc) as tc, ExitStack() as ctx:
        wpool = ctx.enter_context(tc.tile_pool(name="weights", bufs=1))
        xpool = ctx.enter_context(tc.tile_pool(name="xstrips", bufs=2))
        spool = ctx.enter_context(tc.tile_pool(name="staging", bufs=2))
        ppool = ctx.enter_context(tc.tile_pool(name="psum", bufs=4, space="PSUM"))

        # PE warm-up during the initial DMA wait: two interleaved chains of
        # dummy matmuls (one per 64-row tile) on constant SBUF data (non-zero,
        # so the activity monitor sees real switching). Their PSUM tiles come
        # from the same pool tags as the real groups, so the banks are
        # recycled once the dummies are long done.
        dummy = wpool.tile([128, 512], F16, tag="dummy")
        nc.vector.memset(dummy[:, :], 1.0)
        dps0 = ppool.tile([O, 4, W], F32, tag="ps0")
        dps1 = ppool.tile([O, 4, W], F32, tag="ps1")
        for _ in range(12):
            nc.tensor.matmul(dps0[:, :, :], dummy[0:64, 0:O],
                             dummy[0:64, 0:4 * W], start=True, stop=True)
            nc.tensor.matmul(dps1[:, :, :], dummy[64:128, 0:O],
                             dummy[64:128, 0:4 * W], start=True, stop=True)

        # All 9 filter taps in one [128, 9, O] tile, one DMA per half on the
        # scalar HWDGE ring (x loads use the sync ring concurrently).
        wt = wpool.tile([128, 9, O], F16, tag="wt")
        wflat = w_d[:, :, :, :]
        nc.scalar.dma_start(wt[0:64, :, :], wflat)
        nc.scalar.dma_start(wt[64:128, :, :], wflat)

        taps = [(dh, dw) for dh in range(3) for dw in range(3)]

        def load_pair(s, split=False):
            h0 = s * S
            xb = xpool.tile([128, R, W], F16, tag="xs")
            r_lo = 1 if s == 0 else 0
            r_hi = R - 1 if s == NS - 1 else R
            if split:
                # land the first groups' rows early so compute starts sooner
                for a, b in ((r_lo, 8), (8, 20), (20, r_hi)):
                    nc.sync.dma_start(
                        xb[:, a:b, :],
                        x_d[:, :, h0 + a - 1:h0 + b - 1, :],
                    )
            else:
                nc.sync.dma_start(
                    xb[:, r_lo:r_hi, :],
                    x_d[:, :, h0 + r_lo - 1:h0 + r_hi - 1, :],
                )
            return xb

        def compute_pair(s, xb):
            h0 = s * S
            stg0 = spool.tile([O, S, W], F16, tag="stg0")
            stg1 = spool.tile([O, S, W], F16, tag="stg1")
            for j in range(S // 4):
                l0 = 4 * j
                ps0 = ppool.tile([O, 4, W], F32, tag="ps0")
                ps1 = ppool.tile([O, 4, W], F32, tag="ps1")
                for t, (dh, dw) in enumerate(taps):
                    st = t == 0
                    sp = t == 8
                    # output cols valid for this dw (x col = out col + dw - 1)
                    oc0, oc1 = (1, W) if dw == 0 else (0, W - 1) if dw == 2 else (0, W)
                    xc0 = oc0 + dw - 1
                    # output rows valid for this dh (tile row = out row + dh)
                    i0 = 1 if (s == 0 and l0 + dh < 1) else 0
                    i1 = min(4, (R - 1) - l0 - dh) if s == NS - 1 else 4
                    nc.tensor.matmul(
                        ps0[:, i0:i1, oc0:oc1], wt[0:64, t, :],
                        xb[0:64, l0 + dh + i0:l0 + dh + i1, xc0:xc0 + oc1 - oc0],
                        start=st, stop=sp,
                    )
                    nc.tensor.matmul(
                        ps1[:, i0:i1, oc0:oc1], wt[64:128, t, :],
                        xb[64:128, l0 + dh + i0:l0 + dh + i1, xc0:xc0 + oc1 - oc0],
                        start=st, stop=sp,
                    )
                nc.vector.tensor_copy(stg0[:, l0:l0 + 4, :], ps0[:, :, :])
                nc.scalar.copy(stg1[:, l0:l0 + 4, :], ps1[:, :, :])
                # stream out finished rows while the rest computes
                if j == 2:
                    nc.sync.dma_start(o_d[0, :, h0:h0 + 12, :],
                                      stg0[:, 0:12, :])
                    nc.scalar.dma_start(o_d[1, :, h0:h0 + 12, :],
                                        stg1[:, 0:12, :])
                elif j == 5:
                    nc.sync.dma_start(o_d[0, :, h0 + 12:h0 + 24, :],
                                      stg0[:, 12:24, :])
                    nc.scalar.dma_start(o_d[1, :, h0 + 12:h0 + 24, :],
                                        stg1[:, 12:24, :])
            nc.sync.dma_start(o_d[0, :, h0 + 24:h0 + S, :], stg0[:, 24:S, :])
            nc.scalar.dma_start(o_d[1, :, h0 + 24:h0 + S, :], stg1[:, 24:S, :])

        cur = load_pair(0, split=True)
        for s in range(NS):
            nxt = load_pair(s + 1) if s + 1 < NS else None
            compute_pair(s, cur)
            cur = nxt

    nc.compile()
    return nc


def kernel(x: np.ndarray, weight: np.ndarray) -> np.ndarray:
    from concourse.bass_utils import run_bass_kernel_spmd

    if "nc" not in _cache:
        _cache["nc"] = _build_nc()
    nc = _cache["nc"]

    x16 = np.ascontiguousarray(np.asarray(x).astype(np.float16))
    w16 = np.ascontiguousarray(np.asarray(weight).astype(np.float16))

    in_maps = [
        {"x": x16[i * B_LOC:(i + 1) * B_LOC], "weight": w16}
        for i in range(N_CORES)
    ]
    res = run_bass_kernel_spmd(nc, in_maps, list(range(N_CORES)))
    return np.concatenate(
        [np.asarray(res.results[i]["out"], dtype=np.float32)
         for i in range(N_CORES)],
        axis=0,
    )
